# revision 36
# baseline (speedup 1.0000x reference)
"""Causal multi-head attention block (16 heads, dim 1024) on 8 TRN2 NeuronCores.

Sharding: tensor-parallel over heads - core c computes heads {2c, 2c+1}:
  q/k/v projections with the 128-column weight slices, causal attention,
  and a partial output projection with the matching 128 Wout rows.
Host sums the 8 partial outputs and adds the bias.

v2 design notes (vs the transpose-heavy v1):
  * The host supplies x PRE-TRANSPOSED (xT [dim, b*n], bf16). PE-mode
    transposes measured ~275ns each; v1 spent ~140us/core on them. Now the
    kernel contains ZERO PE transposes.
  * q/k are computed feature-major (qT/kT [feat, tok]) for the score
    matmuls: lhsT = W slice, rhs = xT chunk (N=512 streams).
  * v is computed TOKEN-major directly: lhsT = xT tile (stationary),
    rhs = Wv slice -> v[tok, feat] psum, copied into vaug ([128 j, 65] per
    j-tile per head: 64 v cols + a ones col for the softmax denominator).
  * scores TRANSPOSED: dotsT[j,i] = kT.T @ qT per (j-tile, head), K=64 ->
    the two heads go to PE row groups (0,0)/(64,0) and run concurrently.
    exp on ACT (scale=1/32 folded in); diagonal tiles zeroed above the
    diagonal with gpsimd affine_select; AV: outT = vaug.T @ attnT
    accumulated over j-tiles, psum row 64 = softmax denominators.
  * normalize (DVE reciprocal + gpsimd partition_broadcast + DVE stt),
    then output projection per 128-token tile; psum->sbuf copies split
    DVE/gpsimd; DMA out fp16 partials.

Engines run their instruction streams IN ORDER, so phase12(b+1) emission is
interleaved with phase34(b) (software pipelining at emission order) to keep
PE fed while ACT works through the exp stream.
"""
import numpy as np
import ml_dtypes
from contextlib import ExitStack, nullcontext

import concourse.bacc as bacc
import concourse.mybir as mybir
import concourse.tile as tile
import concourse.bass_utils as bass_utils

F32 = mybir.dt.float32
BF16 = mybir.dt.bfloat16
FP16 = mybir.dt.float16

B = 4            # batches
T = 2048         # tokens per batch
DIM = 1024
NT = T // 128    # token tiles per batch (16)
KT = DIM // 128  # contraction tiles (8)
NCHUNK = T // 512  # 512-col i-chunks per batch (4)
SCALE = DIM ** -0.5  # 1/32 - NOTE: full dim, not head dim (matches reference)

TRACE = False
LAST_EXEC_NS = None
LAST_TRACE = None
LAST_PROFILE = None
_CACHED = {}


def build_kernel(nbatches=None, interleave=True):
    NB = nbatches if nbatches is not None else B

    nc = bacc.Bacc("TRN2", target_bir_lowering=False, debug=False, num_devices=8)

    xT_d = nc.dram_tensor("xt", [DIM, B * T], BF16, kind="ExternalInput").ap()
    wq_d = nc.dram_tensor("wq", [DIM, 128], BF16, kind="ExternalInput").ap()
    wk_d = nc.dram_tensor("wk", [DIM, 128], BF16, kind="ExternalInput").ap()
    wv_d = nc.dram_tensor("wv", [DIM, 128], BF16, kind="ExternalInput").ap()
    wo_d = nc.dram_tensor("wo", [128, DIM], BF16, kind="ExternalInput").ap()
    out_d = nc.dram_tensor("out", [B * T, DIM], FP16, kind="ExternalOutput").ap()

    with tile.TileContext(nc) as tc, ExitStack() as ctx:
        cp = ctx.enter_context(tc.tile_pool(name="const", bufs=1))
        xT_p = ctx.enter_context(tc.tile_pool(name="xT", bufs=2))
        qT_p = ctx.enter_context(tc.tile_pool(name="qT", bufs=2))
        kT_p = ctx.enter_context(tc.tile_pool(name="kT", bufs=2))
        vaug_p = ctx.enter_context(tc.tile_pool(name="vaug", bufs=2))
        attnT_p = ctx.enter_context(tc.tile_pool(name="attnT", bufs=4))
        recip_p = ctx.enter_context(tc.tile_pool(name="recip", bufs=4))
        rbc_p = ctx.enter_context(tc.tile_pool(name="rbc", bufs=2))
        outT_p = ctx.enter_context(tc.tile_pool(name="outT", bufs=2))
        osb_p = ctx.enter_context(tc.tile_pool(name="osb", bufs=3))
        mm_ps = ctx.enter_context(tc.tile_pool(name="mmps", bufs=2, space="PSUM"))
        dots_ps = ctx.enter_context(tc.tile_pool(name="dotsps", bufs=1, space="PSUM"))
        av_ps_p = ctx.enter_context(tc.tile_pool(name="avps", bufs=2, space="PSUM"))

        # ---- constants ----
        ones32 = cp.tile([128, NT], F32, tag="ones32")
        nc.gpsimd.memset(ones32[:], 1.0)

        # ---- weights ----
        wq_sb = cp.tile([128, KT * 128], BF16, tag="wq")
        wk_sb = cp.tile([128, KT * 128], BF16, tag="wk")
        wv_sb = cp.tile([128, KT * 128], BF16, tag="wv")
        wo_sb = cp.tile([128, DIM], BF16, tag="wo")
        for w_sb, w_d in ((wq_sb, wq_d), (wk_sb, wk_d), (wv_sb, wv_d)):
            nc.sync.dma_start(w_sb[:].rearrange("p (kt m) -> p kt m", kt=KT),
                              w_d.rearrange("(kt p) m -> p kt m", p=128))
        nc.sync.dma_start(wo_sb[:], wo_d)

        # PE warm-up: ~15 dummy matmuls on uninitialized SBUF overlap the
        # first xT DMA and push the PE p-state to full clock before the
        # first real projection
        scratch = cp.tile([128, 512], BF16, tag="scratch")
        nc.gpsimd.memset(scratch[:], 1.0)
        wps = mm_ps.tile([128, 512], F32, tag="mm", name="warm")
        for _ in range(15):
            nc.tensor.matmul(wps[:], scratch[:, 0:128], scratch[:],
                             start=True, stop=True)

        state = {}  # per-batch qT/kT/vaug handles

        def phase12_steps(b):
            """xT DMA + q/k (feat-major) + v (token-major) for batch b."""
            t0 = b * T
            xT = xT_p.tile([128, KT * T], BF16, tag="xT", name="xT")
            xTv = xT[:].rearrange("p (kt t) -> p kt t", kt=KT)
            xsrc = xT_d.rearrange("(kt p) t -> p kt t", p=128)

            def dma_slice(ch):
                nc.sync.dma_start(xTv[:, :, ch * 512:(ch + 1) * 512],
                                  xsrc[:, :, t0 + ch * 512: t0 + (ch + 1) * 512])

            # keep the DMA 2 token-slices ahead of the projections
            dma_slice(0)
            dma_slice(1)
            yield
            qT = qT_p.tile([128, T], BF16, tag="qT", name="qT")
            kTt = kT_p.tile([128, T], BF16, tag="kT", name="kT")
            vaug = vaug_p.tile([128, NT * 130], BF16, tag="vaug", name="vaug")
            vv = vaug[:].rearrange("p (jt c) -> p jt c", c=130)
            for ch in range(NCHUNK):
                if ch + 2 < NCHUNK:
                    dma_slice(ch + 2)
                # q and k projections for this 512-token chunk; yield
                # mid-accumulation so phase12 PE work spreads evenly across
                # the interleaved phase34 steps
                for w_sb, dest in ((wq_sb, qT), (wk_sb, kTt)):
                    pp = mm_ps.tile([128, 512], F32, tag="mm", name="pp")
                    for kt in range(KT):
                        nc.tensor.matmul(
                            pp[:], w_sb[:, kt * 128:(kt + 1) * 128],
                            xTv[:, kt, ch * 512:(ch + 1) * 512],
                            start=(kt == 0), stop=(kt == KT - 1))
                        if kt == 3:
                            yield
                    nc.vector.tensor_copy(dest[:, ch * 512:(ch + 1) * 512], pp[:])
                    yield
                # v for the same 4 token-tiles, token-major
                vp = mm_ps.tile([128, 512], F32, tag="mm", name="vp")
                for j in range(4):
                    tt = 4 * ch + j
                    for kt in range(KT):
                        nc.tensor.matmul(
                            vp[:, j * 128:(j + 1) * 128],
                            xTv[:, kt, tt * 128:(tt + 1) * 128],
                            wv_sb[:, kt * 128:(kt + 1) * 128],
                            start=(kt == 0), stop=(kt == KT - 1))
                    if j == 1:
                        yield
                src = vp[:].rearrange("p (j c) -> p j c", j=4)
                nc.vector.tensor_copy(vv[:, 4 * ch:4 * ch + 4, 0:64], src[:, :, 0:64])
                nc.vector.tensor_copy(vv[:, 4 * ch:4 * ch + 4, 65:129], src[:, :, 64:128])
                yield
            nc.vector.tensor_copy(
                vaug[:].rearrange("p (u c) -> p u c", c=130)[:, :, 64:65],
                ones32[:].rearrange("p (u o) -> p u o", o=1))
            nc.vector.tensor_copy(
                vaug[:].rearrange("p (u c) -> p u c", c=130)[:, :, 129:130],
                ones32[:].rearrange("p (u o) -> p u o", o=1))
            state[b] = (qT, kTt, vaug)

        def phase34_steps(b):
            """Attention + deferred output projection for batch b.

            The oproj of chunk c is emitted interleaved into chunk c+1's
            pair loop, so the PE never waits on the normalize chain."""
            t0 = b * T
            qT, kTt, vaug = state.pop(b)
            outT = outT_p.tile([128, T], BF16, tag="outT", name="outT")
            deferred = []  # token-tiles whose oproj is pending

            def emit_oproj(tt):
                osb = osb_p.tile([128, DIM], FP16, tag="osb", name="osb")
                for half in (0, 1):
                    po = mm_ps.tile([128, 512], F32, tag="mm", name="po")
                    nc.tensor.matmul(po[:], outT[:, tt * 128:(tt + 1) * 128],
                                     wo_sb[:, half * 512:(half + 1) * 512],
                                     start=True, stop=True)
                    nc.vector.tensor_copy(osb[:, half * 512:(half + 1) * 512],
                                          po[:])
                nc.sync.dma_start(out_d[t0 + tt * 128: t0 + (tt + 1) * 128, :],
                                  osb[:])

            for c in range(NCHUNK):
                njt = 4 * (c + 1)
                avp = {h: av_ps_p.tile([65, 512], F32, tag="av", name=f"avp{h}")
                       for h in (0, 1)}

                def emit_av(pend, njt=njt, avp=avp):
                    jts, offs, at = pend
                    for h in (0, 1):
                        for j, jt in enumerate(jts):
                            off = offs[j]
                            nc.tensor.matmul(
                                avp[h][:, off:512],
                                vaug[:, jt * 130 + 65 * h: jt * 130 + 65 * h + 65],
                                at[:, h * 1024 + j * 512 + off: h * 1024 + (j + 1) * 512],
                                start=(jt == 0), stop=(jt == njt - 1))

                pend = None  # exp'd pair awaiting its AV (one-pair software pipeline)
                for jp in range(njt // 2):
                    jts = (2 * jp, 2 * jp + 1)
                    offs = [max(512 * c, jt * 128) - 512 * c for jt in jts]
                    # AV of the previous pair first: its exp finished during the
                    # last step, so the PE is never parked behind the ACT engine
                    if pend is not None:
                        emit_av(pend)
                    # one psum tile per pair: h0 cols [0:1024], h1 [1024:2048]
                    dps = dots_ps.tile([128, 2048], F32, tag="dots", name="dp")
                    # j outer, h inner: adjacent matmuls hit disjoint PE row
                    # groups (h0 rows 0-63, h1 rows 64-127) and run concurrently
                    for j, jt in enumerate(jts):
                        off = offs[j]
                        for h in (0, 1):
                            nc.tensor.matmul(
                                dps[:, h * 1024 + j * 512 + off: h * 1024 + (j + 1) * 512],
                                kTt[64 * h:64 * h + 64, jt * 128:(jt + 1) * 128],
                                qT[64 * h:64 * h + 64, 512 * c + off:512 * (c + 1)],
                                start=True, stop=True)
                    # single exp over both heads' scores
                    at = attnT_p.tile([128, 2048], BF16, tag="at", name="at")
                    nc.scalar.activation(at[:, offs[0]:2048], dps[:, offs[0]:2048],
                                         mybir.ActivationFunctionType.Exp,
                                         bias=0.0, scale=float(SCALE))
                    for h in (0, 1):
                        for j, jt in enumerate(jts):
                            if jt >= 4 * c:  # zero invalid (j > i) entries
                                base = h * 1024 + j * 512 + offs[j]
                                nc.gpsimd.affine_select(
                                    out=at[:, base: base + 128],
                                    in_=at[:, base: base + 128],
                                    compare_op=mybir.AluOpType.is_ge, fill=0.0,
                                    base=0, pattern=[[1, 128]], channel_multiplier=-1)
                    # pop deferred oproj in the LATE half of each chunk: the
                    # normalize chain behind it is done, phase12 interleave
                    # has thinned out there, and the extra PE work covers the
                    # exp-bound pairs
                    if 2 * jp >= njt // 2:
                        for _ in range(2):
                            if deferred:
                                emit_oproj(deferred.pop(0))
                    pend = (jts, offs, at)
                    yield
                emit_av(pend)
                # normalize immediately (fast recip keeps this short); the
                # dependent oproj is deferred into the next chunk's pairs
                for h in (0, 1):
                    # stage the denominator row into SBUF: the custom-DVE
                    # approx reciprocal mis-reads PSUM operands
                    den = recip_p.tile([1, 512], F32, tag="den", name="den")
                    nc.vector.tensor_copy(den[:], avp[h][64:65, :])
                    rc = recip_p.tile([1, 512], F32, tag="recip", name="rc")
                    nc.vector.reciprocal_approx_fast(rc[:], den[:])
                    rb = rbc_p.tile([64, 512], F32, tag="rbc", name="rb")
                    nc.gpsimd.partition_broadcast(rb[:], rc[:])
                    nc.vector.scalar_tensor_tensor(
                        outT[64 * h:64 * h + 64, c * 512:(c + 1) * 512],
                        avp[h][0:64, :], 1.0, rb[:],
                        op0=mybir.AluOpType.mult, op1=mybir.AluOpType.mult)
                deferred.extend(range(4 * c, 4 * c + 4))
                yield
            while deferred:
                emit_oproj(deferred.pop(0))
                yield

        def drive(gens):
            """Round-robin the emission generators until all are exhausted."""
            gens = [g for g in gens if g is not None]
            while gens:
                nxt = []
                for g in gens:
                    try:
                        next(g)
                        nxt.append(g)
                    except StopIteration:
                        pass
                gens = nxt

        if interleave:
            for b in range(NB + 1):
                drive([phase12_steps(b) if b < NB else None,
                       phase34_steps(b - 1) if b >= 1 else None])
        else:
            for b in range(NB):
                drive([phase12_steps(b)])
                drive([phase34_steps(b)])

    nc.compile()
    return nc


def kernel(x, Wq, Wkv, Wout, bout):
    """Full inputs -> full output. Shards across 8 NeuronCores internally."""
    global LAST_EXEC_NS, LAST_TRACE
    if "nc" not in _CACHED:
        _CACHED["nc"] = build_kernel()
    nc = _CACHED["nc"]

    hdt = ml_dtypes.bfloat16
    xf = np.asarray(x, dtype=np.float32).reshape(B * T, DIM)
    xT = np.ascontiguousarray(xf.T).astype(hdt)  # [DIM, B*T]
    Wq = np.asarray(Wq, dtype=np.float32).astype(hdt)
    Wkv = np.asarray(Wkv, dtype=np.float32).astype(hdt)
    Wout = np.asarray(Wout, dtype=np.float32).astype(hdt)
    bout = np.asarray(bout, dtype=np.float32)

    in_maps = []
    for c in range(8):
        s = slice(128 * c, 128 * (c + 1))
        in_maps.append({
            "xt": xT,
            "wq": np.ascontiguousarray(Wq[:, s]),
            "wk": np.ascontiguousarray(Wkv[:, :DIM][:, s]),
            "wv": np.ascontiguousarray(Wkv[:, DIM:][:, s]),
            "wo": np.ascontiguousarray(Wout[s, :]),
        })

    res = bass_utils.run_bass_kernel_spmd(nc, in_maps, core_ids=list(range(8)),
                                          trace=TRACE)
    if TRACE:
        LAST_EXEC_NS = res.exec_time_ns
        LAST_TRACE = res.instructions_and_trace
        globals()["LAST_PROFILE"] = getattr(res, "profile_json", None)
    acc = res.results[0]["out"].astype(np.float64)
    for c in range(1, 8):
        acc += res.results[c]["out"]
    out = (acc + bout.astype(np.float64)).astype(np.float32)
    return out.reshape(B, T, DIM)


# revision 39
# speedup vs baseline: 1.0173x; 1.0173x over previous
"""Causal multi-head attention block (16 heads, dim 1024) on 8 TRN2 NeuronCores.

Sharding: tensor-parallel over heads - core c computes heads {2c, 2c+1}:
  q/k/v projections with the 128-column weight slices, causal attention,
  and a partial output projection with the matching 128 Wout rows.
Host sums the 8 partial outputs and adds the bias.

Design notes:
  * The host supplies x PRE-TRANSPOSED (xT [dim, b*n], bf16), so the kernel
    contains ZERO PE-mode transposes (each measured ~275ns; the transpose-
    heavy variant spent ~140us/core on them).
  * q/k are computed feature-major (qT/kT [feat, tok]) for the score
    matmuls: lhsT = W slice, rhs = xT chunk (N=512 streams). v is computed
    TOKEN-major directly: lhsT = xT tile, rhs = Wv slice -> v[tok, feat],
    copied into vaug ([128 j, 65] per j-tile per head: 64 v cols + a ones
    col that makes the AV matmul also produce softmax denominators).
  * scores TRANSPOSED: dotsT[j,i] = kT.T @ qT per (j-tile, head), K=64 ->
    the two heads go to PE row groups (0,0)/(64,0) and run CONCURRENTLY;
    both land in one [128, 2048] psum tile so a single ACT exp op covers a
    j-tile pair x both heads (bigger ACT ops amortize its ~300ns/op fixed
    cost - ACT is within ~25% of being the bottleneck engine).
    Diagonal tiles are zeroed above the diagonal with gpsimd affine_select.
  * Software pipelining at emission order (engines execute their streams
    IN ORDER): the AV matmuls of a pair are emitted one pair LATE so the
    PE is never parked waiting on that pair's exp; the output projection
    of chunk c is deferred into chunk c+2's pair loop so the normalize
    chain (SBUF-staged reciprocal_approx_fast + partition_broadcast + stt;
    exact DVE reciprocal costs 3.3us and stalled everything) is never on
    the PE's critical path; phase12(b+1) emission is interleaved with
    phase34(b) at ~1:1 step granularity to fill ACT-bound stretches.
  * All DMA stays on nc.sync: issuing DMA from scalar/gpsimd blocks those
    engines' in-order queues on the DMA's dependencies (measured -10%).
"""
import numpy as np
import ml_dtypes
from contextlib import ExitStack, nullcontext

import concourse.bacc as bacc
import concourse.mybir as mybir
import concourse.tile as tile
import concourse.bass_utils as bass_utils

F32 = mybir.dt.float32
BF16 = mybir.dt.bfloat16
FP16 = mybir.dt.float16

B = 4            # batches
T = 2048         # tokens per batch
DIM = 1024
NT = T // 128    # token tiles per batch (16)
KT = DIM // 128  # contraction tiles (8)
NCHUNK = T // 512  # 512-col i-chunks per batch (4)
SCALE = DIM ** -0.5  # 1/32 - NOTE: full dim, not head dim (matches reference)

TRACE = False
LAST_EXEC_NS = None
LAST_TRACE = None
LAST_PROFILE = None
_CACHED = {}


def build_kernel(nbatches=None, interleave=True):
    NB = nbatches if nbatches is not None else B

    nc = bacc.Bacc("TRN2", target_bir_lowering=False, debug=False, num_devices=8)

    xT_d = nc.dram_tensor("xt", [DIM, B * T], BF16, kind="ExternalInput").ap()
    wq_d = nc.dram_tensor("wq", [DIM, 128], BF16, kind="ExternalInput").ap()
    wk_d = nc.dram_tensor("wk", [DIM, 128], BF16, kind="ExternalInput").ap()
    wv_d = nc.dram_tensor("wv", [DIM, 128], BF16, kind="ExternalInput").ap()
    wo_d = nc.dram_tensor("wo", [128, DIM], BF16, kind="ExternalInput").ap()
    out_d = nc.dram_tensor("out", [B * T, DIM], FP16, kind="ExternalOutput").ap()

    with tile.TileContext(nc) as tc, ExitStack() as ctx:
        cp = ctx.enter_context(tc.tile_pool(name="const", bufs=1))
        xT_p = ctx.enter_context(tc.tile_pool(name="xT", bufs=2))
        qT_p = ctx.enter_context(tc.tile_pool(name="qT", bufs=2))
        kT_p = ctx.enter_context(tc.tile_pool(name="kT", bufs=2))
        vaug_p = ctx.enter_context(tc.tile_pool(name="vaug", bufs=2))
        attnT_p = ctx.enter_context(tc.tile_pool(name="attnT", bufs=4))
        recip_p = ctx.enter_context(tc.tile_pool(name="recip", bufs=4))
        rbc_p = ctx.enter_context(tc.tile_pool(name="rbc", bufs=2))
        outT_p = ctx.enter_context(tc.tile_pool(name="outT", bufs=2))
        osb_p = ctx.enter_context(tc.tile_pool(name="osb", bufs=3))
        mm_ps = ctx.enter_context(tc.tile_pool(name="mmps", bufs=2, space="PSUM"))
        dots_ps = ctx.enter_context(tc.tile_pool(name="dotsps", bufs=1, space="PSUM"))
        av_ps_p = ctx.enter_context(tc.tile_pool(name="avps", bufs=2, space="PSUM"))

        # ---- constants ----
        ones32 = cp.tile([128, NT], F32, tag="ones32")
        nc.gpsimd.memset(ones32[:], 1.0)

        # ---- weights ----
        wq_sb = cp.tile([128, KT * 128], BF16, tag="wq")
        wk_sb = cp.tile([128, KT * 128], BF16, tag="wk")
        wv_sb = cp.tile([128, KT * 128], BF16, tag="wv")
        wo_sb = cp.tile([128, DIM], BF16, tag="wo")
        for w_sb, w_d in ((wq_sb, wq_d), (wk_sb, wk_d), (wv_sb, wv_d)):
            nc.sync.dma_start(w_sb[:].rearrange("p (kt m) -> p kt m", kt=KT),
                              w_d.rearrange("(kt p) m -> p kt m", p=128))
        nc.sync.dma_start(wo_sb[:], wo_d)

        state = {}  # per-batch qT/kT/vaug handles

        def phase12_steps(b):
            """xT DMA + q/k (feat-major) + v (token-major) for batch b."""
            t0 = b * T
            xT = xT_p.tile([128, KT * T], BF16, tag="xT", name="xT")
            xTv = xT[:].rearrange("p (kt t) -> p kt t", kt=KT)
            xsrc = xT_d.rearrange("(kt p) t -> p kt t", p=128)

            def dma_slice(ch):
                nc.sync.dma_start(xTv[:, :, ch * 512:(ch + 1) * 512],
                                  xsrc[:, :, t0 + ch * 512: t0 + (ch + 1) * 512])

            # keep the DMA 2 token-slices ahead of the projections
            dma_slice(0)
            dma_slice(1)
            yield
            qT = qT_p.tile([128, T], BF16, tag="qT", name="qT")
            kTt = kT_p.tile([128, T], BF16, tag="kT", name="kT")
            vaug = vaug_p.tile([128, NT * 130], BF16, tag="vaug", name="vaug")
            vv = vaug[:].rearrange("p (jt c) -> p jt c", c=130)
            for ch in range(NCHUNK):
                if ch + 2 < NCHUNK:
                    dma_slice(ch + 2)
                # q and k projections for this 512-token chunk; yield
                # mid-accumulation so phase12 PE work spreads evenly across
                # the interleaved phase34 steps
                for w_sb, dest in ((wq_sb, qT), (wk_sb, kTt)):
                    pp = mm_ps.tile([128, 512], F32, tag="mm", name="pp")
                    for kt in range(KT):
                        nc.tensor.matmul(
                            pp[:], w_sb[:, kt * 128:(kt + 1) * 128],
                            xTv[:, kt, ch * 512:(ch + 1) * 512],
                            start=(kt == 0), stop=(kt == KT - 1))
                        if kt == 3:
                            yield
                    nc.vector.tensor_copy(dest[:, ch * 512:(ch + 1) * 512], pp[:])
                    yield
                # v for the same 4 token-tiles, token-major
                vp = mm_ps.tile([128, 512], F32, tag="mm", name="vp")
                for j in range(4):
                    tt = 4 * ch + j
                    for kt in range(KT):
                        nc.tensor.matmul(
                            vp[:, j * 128:(j + 1) * 128],
                            xTv[:, kt, tt * 128:(tt + 1) * 128],
                            wv_sb[:, kt * 128:(kt + 1) * 128],
                            start=(kt == 0), stop=(kt == KT - 1))
                    if j == 1:
                        yield
                src = vp[:].rearrange("p (j c) -> p j c", j=4)
                nc.vector.tensor_copy(vv[:, 4 * ch:4 * ch + 4, 0:64], src[:, :, 0:64])
                nc.vector.tensor_copy(vv[:, 4 * ch:4 * ch + 4, 65:129], src[:, :, 64:128])
                yield
            nc.vector.tensor_copy(
                vaug[:].rearrange("p (u c) -> p u c", c=130)[:, :, 64:65],
                ones32[:].rearrange("p (u o) -> p u o", o=1))
            nc.vector.tensor_copy(
                vaug[:].rearrange("p (u c) -> p u c", c=130)[:, :, 129:130],
                ones32[:].rearrange("p (u o) -> p u o", o=1))
            state[b] = (qT, kTt, vaug)

        def phase34_steps(b):
            """Attention + deferred output projection for batch b.

            The oproj of chunk c is emitted interleaved into chunk c+1's
            pair loop, so the PE never waits on the normalize chain."""
            t0 = b * T
            qT, kTt, vaug = state.pop(b)
            outT = outT_p.tile([128, T], BF16, tag="outT", name="outT")
            deferred = []  # token-tiles whose oproj is pending

            def emit_oproj(tt):
                osb = osb_p.tile([128, DIM], FP16, tag="osb", name="osb")
                for half in (0, 1):
                    po = mm_ps.tile([128, 512], F32, tag="mm", name="po")
                    nc.tensor.matmul(po[:], outT[:, tt * 128:(tt + 1) * 128],
                                     wo_sb[:, half * 512:(half + 1) * 512],
                                     start=True, stop=True)
                    nc.vector.tensor_copy(osb[:, half * 512:(half + 1) * 512],
                                          po[:])
                nc.sync.dma_start(out_d[t0 + tt * 128: t0 + (tt + 1) * 128, :],
                                  osb[:])

            for c in range(NCHUNK):
                njt = 4 * (c + 1)
                avp = {h: av_ps_p.tile([65, 512], F32, tag="av", name=f"avp{h}")
                       for h in (0, 1)}

                def emit_av(pend, njt=njt, avp=avp):
                    jts, offs, at = pend
                    for h in (0, 1):
                        for j, jt in enumerate(jts):
                            off = offs[j]
                            nc.tensor.matmul(
                                avp[h][:, off:512],
                                vaug[:, jt * 130 + 65 * h: jt * 130 + 65 * h + 65],
                                at[:, h * 1024 + j * 512 + off: h * 1024 + (j + 1) * 512],
                                start=(jt == 0), stop=(jt == njt - 1))

                pend = None  # exp'd pair awaiting its AV (one-pair software pipeline)
                for jp in range(njt // 2):
                    jts = (2 * jp, 2 * jp + 1)
                    offs = [max(512 * c, jt * 128) - 512 * c for jt in jts]
                    # AV of the previous pair first: its exp finished during the
                    # last step, so the PE is never parked behind the ACT engine
                    if pend is not None:
                        emit_av(pend)
                    # one psum tile per pair: h0 cols [0:1024], h1 [1024:2048]
                    dps = dots_ps.tile([128, 2048], F32, tag="dots", name="dp")
                    # j outer, h inner: adjacent matmuls hit disjoint PE row
                    # groups (h0 rows 0-63, h1 rows 64-127) and run concurrently
                    for j, jt in enumerate(jts):
                        off = offs[j]
                        for h in (0, 1):
                            nc.tensor.matmul(
                                dps[:, h * 1024 + j * 512 + off: h * 1024 + (j + 1) * 512],
                                kTt[64 * h:64 * h + 64, jt * 128:(jt + 1) * 128],
                                qT[64 * h:64 * h + 64, 512 * c + off:512 * (c + 1)],
                                start=True, stop=True)
                    # single exp over both heads' scores
                    at = attnT_p.tile([128, 2048], BF16, tag="at", name="at")
                    nc.scalar.activation(at[:, offs[0]:2048], dps[:, offs[0]:2048],
                                         mybir.ActivationFunctionType.Exp,
                                         bias=0.0, scale=float(SCALE))
                    for h in (0, 1):
                        for j, jt in enumerate(jts):
                            if jt >= 4 * c:  # zero invalid (j > i) entries
                                base = h * 1024 + j * 512 + offs[j]
                                nc.gpsimd.affine_select(
                                    out=at[:, base: base + 128],
                                    in_=at[:, base: base + 128],
                                    compare_op=mybir.AluOpType.is_ge, fill=0.0,
                                    base=0, pattern=[[1, 128]], channel_multiplier=-1)
                    # lag the oproj ~2 chunks behind so the normalize chain
                    # (recip+bcast+stt on DVE/gpsimd) is long done by then
                    if len(deferred) > 4:
                        emit_oproj(deferred.pop(0))
                    pend = (jts, offs, at)
                    yield
                emit_av(pend)
                # normalize immediately (fast recip keeps this short); the
                # dependent oproj is deferred into the next chunk's pairs
                for h in (0, 1):
                    # stage the denominator row into SBUF: the custom-DVE
                    # approx reciprocal mis-reads PSUM operands
                    den = recip_p.tile([1, 512], F32, tag="den", name="den")
                    nc.vector.tensor_copy(den[:], avp[h][64:65, :])
                    rc = recip_p.tile([1, 512], F32, tag="recip", name="rc")
                    nc.vector.reciprocal_approx_fast(rc[:], den[:])
                    rb = rbc_p.tile([64, 512], F32, tag="rbc", name="rb")
                    nc.gpsimd.partition_broadcast(rb[:], rc[:])
                    nc.vector.scalar_tensor_tensor(
                        outT[64 * h:64 * h + 64, c * 512:(c + 1) * 512],
                        avp[h][0:64, :], 1.0, rb[:],
                        op0=mybir.AluOpType.mult, op1=mybir.AluOpType.mult)
                deferred.extend(range(4 * c, 4 * c + 4))
                yield
            while deferred:
                emit_oproj(deferred.pop(0))
                yield

        def drive(gens):
            """Round-robin the emission generators until all are exhausted."""
            gens = [g for g in gens if g is not None]
            while gens:
                nxt = []
                for g in gens:
                    try:
                        next(g)
                        nxt.append(g)
                    except StopIteration:
                        pass
                gens = nxt

        if interleave:
            for b in range(NB + 1):
                drive([phase12_steps(b) if b < NB else None,
                       phase34_steps(b - 1) if b >= 1 else None])
        else:
            for b in range(NB):
                drive([phase12_steps(b)])
                drive([phase34_steps(b)])

    nc.compile()
    return nc


def kernel(x, Wq, Wkv, Wout, bout):
    """Full inputs -> full output. Shards across 8 NeuronCores internally."""
    global LAST_EXEC_NS, LAST_TRACE
    if "nc" not in _CACHED:
        _CACHED["nc"] = build_kernel()
    nc = _CACHED["nc"]

    hdt = ml_dtypes.bfloat16
    xf = np.asarray(x, dtype=np.float32).reshape(B * T, DIM)
    xT = np.ascontiguousarray(xf.T).astype(hdt)  # [DIM, B*T]
    Wq = np.asarray(Wq, dtype=np.float32).astype(hdt)
    Wkv = np.asarray(Wkv, dtype=np.float32).astype(hdt)
    Wout = np.asarray(Wout, dtype=np.float32).astype(hdt)
    bout = np.asarray(bout, dtype=np.float32)

    in_maps = []
    for c in range(8):
        s = slice(128 * c, 128 * (c + 1))
        in_maps.append({
            "xt": xT,
            "wq": np.ascontiguousarray(Wq[:, s]),
            "wk": np.ascontiguousarray(Wkv[:, :DIM][:, s]),
            "wv": np.ascontiguousarray(Wkv[:, DIM:][:, s]),
            "wo": np.ascontiguousarray(Wout[s, :]),
        })

    res = bass_utils.run_bass_kernel_spmd(nc, in_maps, core_ids=list(range(8)),
                                          trace=TRACE)
    if TRACE:
        LAST_EXEC_NS = res.exec_time_ns
        LAST_TRACE = res.instructions_and_trace
        globals()["LAST_PROFILE"] = getattr(res, "profile_json", None)
    acc = res.results[0]["out"].astype(np.float64)
    for c in range(1, 8):
        acc += res.results[c]["out"]
    out = (acc + bout.astype(np.float64)).astype(np.float32)
    return out.reshape(B, T, DIM)
